# revision 1
# baseline (speedup 1.0000x reference)
import os
import sys

sys.path.insert(0, "/opt/trn_rl_repo")

import numpy as np
import ml_dtypes

import concourse.bass as bass
import concourse.bacc as bacc
import concourse.mybir as mybir
from concourse.bass_utils import run_bass_kernel_spmd
from concourse.tile import TileContext

S = 1024
DIM = 2560
HD = 128
NH = 20
NKV = 5
GS = 128
THETA = 500000.0
EPS = 1e-05
KBASE = NH * HD            # k rows start in w_qkv
VBASE = KBASE + NKV * HD   # v rows start
NC = 8
KCH = DIM // 128           # 20 k-chunks
WQCOLS = 7 * 128           # [qs0 qs1 qs2 kA vA kB vB]
OC = DIM // NC             # 320 output cols per core
MT = S // 128              # 8 token tiles

# head assignment per core: [slot0, slot1, slot2]; None = garbage slot
HEADS = [
    [0, 1, 8], [2, 3, 9], [4, 5, 10], [6, 7, 11],
    [12, 13, None], [14, 15, None], [16, 17, None], [18, 19, None],
]
GA = [0, 0, 1, 1, 3, 3, 4, 4]              # kv group for slots 0,1
GB = [2, 2, 2, 2, None, None, None, None]  # kv group for slot 2
REAL_CHUNKS = [j * 3 + s for j in range(NC) for s in range(3) if HEADS[j][s] is not None]
assert len(REAL_CHUNKS) == NH

FP16 = np.float16
SCALE = float(HD) ** -0.5
ESHIFT = -2.0  # exp(score*SCALE + ESHIFT); cancels in softmax ratio

_cached = {}


def _build_nc():
    nc = bacc.Bacc("TRN2", target_bir_lowering=False, debug=False, num_devices=NC)
    f32 = mybir.dt.float32
    f16 = mybir.dt.float16
    i16 = mybir.dt.int16

    xs_d = nc.declare_dram_parameter("xs", [128, DIM], f32, isOutput=False)
    wq_d = nc.declare_dram_parameter("wq", [DIM, WQCOLS], f16, isOutput=False)
    wo_d = nc.declare_dram_parameter("wo", [NC * 384, OC], f16, isOutput=False)
    tq1_d = nc.declare_dram_parameter("tq1", [S, HD], f16, isOutput=False)
    tq2_d = nc.declare_dram_parameter("tq2", [S, HD], f16, isOutput=False)
    tk1_d = nc.declare_dram_parameter("tk1", [S, HD], f16, isOutput=False)
    tk2_d = nc.declare_dram_parameter("tk2", [S, HD], f16, isOutput=False)
    # 4 causal mask variants for 512-wide score groups: r = kc - 4*grp
    cmask_d = nc.declare_dram_parameter("cmask", [4 * 128, 512], f16, isOutput=False)
    out_d = nc.declare_dram_parameter("out", [S, OC], f32, isOutput=True)

    AQW = DIM + 16  # q8 columns + per-token scale (col DIM) + pad
    agq_in = nc.dram_tensor("agqin", [128, AQW], f16, kind="Internal")
    agq_out = nc.dram_tensor("agqout", [S, AQW], f16, kind="Internal",
                             addr_space="Shared")
    warm_in = nc.dram_tensor("warmin", [16, 16], f16, kind="Internal")
    warm_out = nc.dram_tensor("warmout", [NC * 16, 16], f16, kind="Internal",
                              addr_space="Shared")
    # half A (the tail) is gathered in two pieces: slots 0-1 fire while
    # slot 2 still computes, so o_proj-A's 16-head phase overlaps AG-A2
    agin_a1 = nc.dram_tensor("agina1", [256, S // 2], f16, kind="Internal")
    agin_a2 = nc.dram_tensor("agina2", [128, S // 2], f16, kind="Internal")
    agin_b = nc.dram_tensor("aginb", [384, S // 2], f16, kind="Internal")
    agout_a1 = nc.dram_tensor("agouta1", [NC * 256, S // 2], f16, kind="Internal",
                              addr_space="Shared")
    agout_a2 = nc.dram_tensor("agouta2", [NC * 128, S // 2], f16, kind="Internal",
                              addr_space="Shared")
    agout_b = nc.dram_tensor("agoutb", [NC * 384, S // 2], f16, kind="Internal",
                             addr_space="Shared")
    qrope_d = nc.dram_tensor("qroped", [5 * S, HD], f16, kind="Internal")

    with TileContext(nc) as tc:
        with (
            tc.tile_pool(name="cst", bufs=1) as cst,
            tc.tile_pool(name="kvsb", bufs=1) as kvsb,
            tc.tile_pool(name="nrp", bufs=2) as nrp,
            tc.tile_pool(name="wop", bufs=NH) as wop,
        ):
            ones_row = cst.tile([1, 128], f32, tag="ones", name="ones")
            nc.vector.memset(ones_row[:, :], 1.0)
            ones_col = cst.tile([128, 1], f16, tag="onesc", name="onesc")
            nc.vector.memset(ones_col[:, :], 1.0)
            eshift = cst.tile([128, 1], f32, tag="esh", name="esh")
            nc.vector.memset(eshift[:, :], ESHIFT)

            # Warmup collective: pays the ~30us ncfw cold-start + launch-skew
            # barrier while the quant chain runs, so the real AllGather below
            # enters the mesh hot. No data deps; transfers garbage.
            nc.gpsimd.collective_compute(
                "AllGather", mybir.AluOpType.bypass,
                ins=[warm_in.ap().opt()], outs=[warm_out.ap().opt()],
                replica_groups=[list(range(NC))],
            )

            # ---- Stage A first: sharded quant (own 128 tokens) + AllGather.
            # Issued before all weight/table loads so the quant chain and the
            # collective trigger are at the head of every engine queue.
            with tc.tile_pool(name="xa", bufs=1) as xap:
                xa = xap.tile([128, DIM], f32, tag="x", name="x")
                nc.sync.dma_start(out=xa[:, :], in_=xs_d[:, :])
                mx = xap.tile([128, 1], f32, tag="mx", name="mx")
                nc.vector.tensor_reduce(mx[:, :], xa[:, :],
                                        mybir.AxisListType.X,
                                        mybir.AluOpType.max,
                                        apply_absolute_value=True)
                mx2 = xap.tile([128, 1], f32, tag="mx2", name="mx2")
                nc.vector.tensor_scalar_max(mx2[:, :], mx[:, :], 1e-5)
                rmx = xap.tile([128, 1], f32, tag="rmx", name="rmx")
                nc.vector.reciprocal(rmx[:, :], mx2[:, :])
                s_col = xap.tile([128, 1], f32, tag="scol", name="scol")
                nc.vector.tensor_scalar_mul(s_col[:, :], rmx[:, :], 127.0)
                q16 = xap.tile([128, DIM], i16, tag="q16", name="q16")
                nc.scalar.activation(q16[:, :], xa[:, :],
                                     mybir.ActivationFunctionType.Copy,
                                     scale=s_col[:, 0:1])
                qf = xap.tile([128, AQW], f16, tag="qf", name="qf")
                nc.vector.tensor_copy(qf[:, 0:1280], q16[:, 0:1280])
                nc.scalar.copy(qf[:, 1280:DIM], q16[:, 1280:DIM])
                nc.vector.memset(qf[:, DIM:AQW], 0.0)
                nc.vector.tensor_scalar_mul(qf[:, DIM:DIM + 1], mx2[:, :],
                                            1.0 / 127.0)
                nc.sync.dma_start(out=agq_in[:, :], in_=qf[:, :])
            nc.gpsimd.collective_compute(
                "AllGather", mybir.AluOpType.bypass,
                ins=[agq_in.ap().opt()], outs=[agq_out.ap().opt()],
                replica_groups=[list(range(NC))],
            )

            cmask = cst.tile([128, 4, 512], f16, tag="cm", name="cm")
            nc.sync.dma_start(out=cmask[:, :, :],
                              in_=cmask_d.ap().rearrange("(r p) n -> p r n", p=128))
            # o_proj weights: independent of everything, load early
            wo_sb = []
            for ck in REAL_CHUNKS:
                w = wop.tile([128, OC], f16, tag="wo", name="wo")
                nc.scalar.dma_start(out=w[:, :], in_=wo_d[ck * 128:(ck + 1) * 128, :])
                wo_sb.append(w)

            tabs = {}
            for nm, d in (("tq1", tq1_d), ("tq2", tq2_d),
                          ("tk1", tk1_d), ("tk2", tk2_d)):
                t = cst.tile([128, MT, HD], f16, tag=f"tb{nm}", name=f"tb{nm}")
                nc.sync.dma_start(out=t[:, :, :],
                                  in_=d.ap().rearrange("(m p) d -> p m d", p=128))
                for m in range(MT):
                    tabs[(nm, m)] = t[:, m, :]

            rs_cols = cst.tile([128, MT], f32, tag="rscols", name="rscols")

            KT = [kvsb.tile([128, S], f16, tag=f"KT{b}", name=f"KT{b}") for b in range(2)]
            VV = [[kvsb.tile([128, 128], f16, tag=f"V{b}_{m}", name=f"V{b}_{m}")
                   for m in range(MT)] for b in range(2)]
            qT = [kvsb.tile([128, S], f16, tag=f"qT{s}", name=f"qT{s}") for s in range(3)]

            # rope output staging: [p, 5 slots, MT, d] fp16 (q0,q1,q2,kA,kB)
            rbp = tc.tile_pool(name="rbp", bufs=1)
            rbq = rbp.__enter__().tile([128, 5, MT, HD], f16, tag="rbq", name="rbq")

            def norm_rope_batched(eng, xn_view, t1, t2, ob_view, scratch_tag):
                """xn_view [128, nh, 128] f16 normalized input in d-permuted
                layout (even dims in cols 0:64, odd in 64:128); t1/t2 f16
                split tables [128, 128]; writes roped f16 [128, nh, 128] in
                the same permuted layout. Contiguous step-1 slices keep the
                DVE in 2x 16-bit mode."""
                nh = xn_view.shape[1]
                x0 = xn_view[:, :, 0:64]
                x1 = xn_view[:, :, 64:128]
                t1b = t1.rearrange("p (one d) -> p one d", one=1).to_broadcast(
                    [128, nh, HD])
                t2b = t2.rearrange("p (one d) -> p one d", one=1).to_broadcast(
                    [128, nh, HD])
                a1 = nrp.tile([128, nh, 64], f16, tag=f"ra1{scratch_tag}",
                              name=f"ra1{scratch_tag}")
                a2 = nrp.tile([128, nh, 64], f16, tag=f"ra2{scratch_tag}",
                              name=f"ra2{scratch_tag}")
                eng.tensor_mul(a1[:, :, :], x0, t1b[:, :, 0:64])
                eng.tensor_mul(a2[:, :, :], x1, t2b[:, :, 64:128])
                eng.tensor_sub(ob_view[:, :, 0:64], a1[:, :, :], a2[:, :, :])
                eng.tensor_mul(a1[:, :, :], x0, t2b[:, :, 0:64])
                eng.tensor_mul(a2[:, :, :], x1, t1b[:, :, 64:128])
                eng.tensor_add(ob_view[:, :, 64:128], a1[:, :, :], a2[:, :, :])

            with (
                tc.tile_pool(name="wqp", bufs=KCH) as wqp,
                tc.tile_pool(name="qtp", bufs=KCH) as qtp,
            ):
                wq_sb = []
                for kc in range(KCH):
                    t = wqp.tile([128, WQCOLS], f16, tag="wq", name="wq")
                    nc.scalar.dma_start(out=t[:, :], in_=wq_d[kc * 128:(kc + 1) * 128, :])
                    wq_sb.append(t)

                q8T = [qtp.tile([128, S], f16, tag="q8T", name="q8T", uniquify=True)
                       for _ in range(KCH)]

                # per-token 1/s for all 1024 tokens: [tok%128, tok//128]
                with nc.allow_non_contiguous_dma(reason="2KB scale col gather"):
                    rsg = cst.tile([128, MT], f16, tag="rsg", name="rsg")
                    nc.sync.dma_start(
                        out=rsg[:, :],
                        in_=agq_out.ap()[:, DIM:DIM + 1]
                        .rearrange("(m p) one -> p (m one)", p=128))
                nc.vector.tensor_copy(rs_cols[:, :], rsg[:, :])
                # transposed reload; stage-C half 0 chases these kc-by-kc
                for kc in range(KCH):
                    nc.sync.dma_start(out=q8T[kc][:, :],
                                      in_=agq_out[:, kc * 128:(kc + 1) * 128],
                                      transpose=True)

                # ---- Stage C: qkv matmul + epilogues ----
                with tc.tile_pool(name="psq", bufs=4, space="PSUM") as psq:
                    # half 0: kc-outer so each q8T chunk is consumed on arrival
                    psAh = [psq.tile([128, 384], f32, tag="psA", name="psA")
                            for _ in range(4)]
                    psBh = [psq.tile([128, 512], f32, tag="psB", name="psB")
                            for _ in range(4)]
                    for kc in range(KCH):
                        for m in range(4):
                            lh = q8T[kc][:, m * 128:(m + 1) * 128]
                            nc.tensor.matmul(psAh[m][:, :], lh, wq_sb[kc][:, 0:384],
                                             start=(kc == 0), stop=(kc == KCH - 1))
                            nc.tensor.matmul(psBh[m][:, :], lh, wq_sb[kc][:, 384:896],
                                             start=(kc == 0), stop=(kc == KCH - 1))
                    for m in range(MT):
                        if m < 4:
                            psA, psB = psAh[m], psBh[m]
                        else:
                            psA = psq.tile([128, 384], f32, tag="psA", name="psA")
                            psB = psq.tile([128, 512], f32, tag="psB", name="psB")
                            for kc in range(KCH):
                                lh = q8T[kc][:, m * 128:(m + 1) * 128]
                                nc.tensor.matmul(psA[:, :], lh, wq_sb[kc][:, 0:384],
                                                 start=(kc == 0), stop=(kc == KCH - 1))
                                nc.tensor.matmul(psB[:, :], lh, wq_sb[kc][:, 384:896],
                                                 start=(kc == 0), stop=(kc == KCH - 1))
                        rs_ap = rs_cols[:, m:m + 1]
                        # evacuate q (3 heads) and k (2 heads) f32; V scaled fp16
                        qxs = nrp.tile([128, 384], f32, tag="qxs", name="qxs")
                        nc.scalar.copy(qxs[:, :], psA[:, :])
                        kxs = nrp.tile([128, 2, 128], f32, tag="kxs", name="kxs")
                        nc.scalar.copy(kxs[:, :, :],
                                       psB.rearrange("p (b c) -> p b c", c=256)[:, :, 0:128])
                        for blk in range(2):
                            nc.scalar.activation(VV[blk][m][:, :],
                                                 psB[:, blk * 256 + 128:blk * 256 + 256],
                                                 mybir.ActivationFunctionType.Copy,
                                                 scale=rs_ap)
                        # rms factors for all 5 heads in one [128, 5] tile
                        sq = nrp.tile([128, 384], f32, tag="sqq", name="sqq")
                        sk = nrp.tile([128, 256], f32, tag="sqk", name="sqk")
                        nc.vector.tensor_mul(sq[:, :], qxs[:, :], qxs[:, :])
                        nc.vector.tensor_mul(sk[:, :], kxs.rearrange("p b c -> p (b c)"),
                                               kxs.rearrange("p b c -> p (b c)"))
                        rs5 = nrp.tile([128, 5], f32, tag="rs5", name="rs5")
                        nc.vector.tensor_reduce(rs5[:, 0:3],
                                                sq.rearrange("p (h d) -> p h d", d=128),
                                                mybir.AxisListType.X, mybir.AluOpType.add)
                        nc.vector.tensor_reduce(rs5[:, 3:5],
                                                sk.rearrange("p (h d) -> p h d", d=128),
                                                mybir.AxisListType.X, mybir.AluOpType.add)
                        nc.vector.tensor_scalar(rs5[:, :], rs5[:, :], 1.0 / HD, EPS,
                                                mybir.AluOpType.mult, mybir.AluOpType.add)
                        nc.vector.reciprocal(rs5[:, :], rs5[:, :])
                        nc.scalar.activation(rs5[:, :], rs5[:, :],
                                             mybir.ActivationFunctionType.Sqrt)
                        # normalize (per-head per-partition scalar) then rope
                        for h in range(3):
                            nc.vector.tensor_scalar_mul(qxs[:, h * 128:(h + 1) * 128],
                                                        qxs[:, h * 128:(h + 1) * 128],
                                                        rs5[:, h:h + 1])
                        for h in range(2):
                            nc.vector.tensor_scalar_mul(kxs[:, h, :], kxs[:, h, :],
                                                        rs5[:, 3 + h:4 + h])
                        qx16 = nrp.tile([128, 384], f16, tag="qx16", name="qx16")
                        nc.vector.tensor_copy(qx16[:, :], qxs[:, :])
                        kx16 = nrp.tile([128, 2, 128], f16, tag="kx16",
                                        name="kx16")
                        nc.gpsimd.tensor_copy(kx16[:, :, :], kxs[:, :, :])
                        norm_rope_batched(nc.vector,
                                          qx16.rearrange("p (h d) -> p h d", d=128),
                                          tabs[("tq1", m)], tabs[("tq2", m)],
                                          rbq[:, 0:3, m, :], "q")
                        norm_rope_batched(nc.gpsimd, kx16[:, :, :],
                                          tabs[("tk1", m)], tabs[("tk2", m)],
                                          rbq[:, 3:5, m, :], "k")
                        # stream this m-tile's roped heads out now so only the
                        # transposed reloads remain after the last tile
                        with nc.allow_non_contiguous_dma(reason="rope 256B rows"):
                            nc.sync.dma_start(
                                out=qrope_d.ap().rearrange(
                                    "(i m p) d -> p i m d", i=5, p=128)[:, :, m, :],
                                in_=rbq[:, :, m, :])

            # transposed reloads, ordered so slot-0 attention starts earliest:
            # KT[0] (slots 0/1), qT[0], qT[1], KT[1] (slot 2), qT[2]
            for i in (3, 0, 1, 4, 2):
                dst = qT[i] if i < 3 else KT[i - 3]
                nc.sync.dma_start(out=dst[:, :], in_=qrope_d[i * S:(i + 1) * S, :],
                                  transpose=True)
            rbp.__exit__(None, None, None)

            # ---- Stage F: attention, 512-wide q groups, scoresT [k, q] ----
            # softmax denom per (grp, slot) via ones-column matmul; attention
            # output normalized by 1/den and AllGathered in fp16 (o_proj is
            # computed unquantized — the reference's act-quant scales cancel
            # to within the rel-err budget). o_proj PSUM pool is concurrent
            # so half-A o_proj overlaps group-1 attention.
            with (
                tc.tile_pool(name="pssc", bufs=2, space="PSUM") as pssc,
                tc.tile_pool(name="psav", bufs=2, space="PSUM") as psav,
                tc.tile_pool(name="psden", bufs=1, space="PSUM") as psden,
                tc.tile_pool(name="psbc", bufs=1, space="PSUM") as psbc,
                tc.tile_pool(name="pso", bufs=2, space="PSUM") as pso,
                tc.tile_pool(name="ptt", bufs=12) as ptt,
                tc.tile_pool(name="accp", bufs=3) as accp,
                tc.tile_pool(name="qga", bufs=4) as qga,
                tc.tile_pool(name="agtp", bufs=2 * NH) as agtp,
                tc.tile_pool(name="ogp", bufs=2) as ogp,
            ):
                for grp in (1, 0):
                    gs = slice(grp * 512, grp * 512 + 512)
                    nkc = 4 * grp + 4
                    for sl in range(3):
                        blk = 0 if sl < 2 else 1
                        pts = []
                        den_ps = psden.tile([1, 512], f32, tag="den", name="den")
                        for kc in range(nkc):
                            ps = pssc.tile([128, 512], f32, tag="sc", name="sc")
                            nc.tensor.matmul(ps[:, :],
                                             KT[blk][:, kc * 128:(kc + 1) * 128],
                                             qT[sl][:, gs], start=True, stop=True)
                            pt = ptt.tile([128, 512], f16, tag="pt", name="pt")
                            nc.scalar.activation(pt[:, :], ps[:, :],
                                                 mybir.ActivationFunctionType.Exp,
                                                 bias=eshift[:, 0:1], scale=SCALE)
                            r = kc - 4 * grp
                            if r >= 0:
                                nc.vector.tensor_mul(pt[:, :], pt[:, :], cmask[:, r, :])
                            pts.append(pt)
                            nc.tensor.matmul(den_ps[:, :], ones_col[:, :], pt[:, :],
                                             start=(kc == 0), stop=(kc == nkc - 1))
                        avp = psav.tile([128, 512], f32, tag="av", name="av")
                        for kc in range(nkc):
                            nc.tensor.matmul(avp[:, :], VV[blk][kc][:, :], pts[kc][:, :],
                                             start=(kc == 0), stop=(kc == nkc - 1))
                        # 1/den = exp(-ln(den)) via scalar LUTs + a DVE negate;
                        # the DVE iterative-divide reciprocal costs ~3.4us for
                        # 512 elems on a single partition
                        lnd = accp.tile([1, 512], f32, tag="lnd", name="lnd")
                        nc.scalar.activation(lnd[:, :], den_ps[:, :],
                                             mybir.ActivationFunctionType.Ln)
                        nld = accp.tile([1, 512], f32, tag="nld", name="nld")
                        nc.vector.tensor_scalar_mul(nld[:, :], lnd[:, :], -1.0)
                        rden_row = accp.tile([1, 512], f32, tag="rdr", name="rdr")
                        nc.scalar.activation(rden_row[:, :], nld[:, :],
                                             mybir.ActivationFunctionType.Exp)
                        bp = psbc.tile([128, 512], f32, tag="bc", name="bc")
                        nc.tensor.matmul(bp[:, :], ones_row[:, :], rden_row[:, :],
                                         start=True, stop=True)
                        fac = qga.tile([128, 512], f32, tag="fac", name="fac")
                        nc.scalar.copy(fac[:, :], bp[:, :])
                        aq = qga.tile([128, 512], f16, tag="aq", name="aq")
                        nc.vector.tensor_mul(aq[:, :], avp[:, :], fac[:, :])
                        if grp == 1:
                            nc.sync.dma_start(
                                out=agin_b[sl * 128:(sl + 1) * 128, :],
                                in_=aq[:, :])
                        elif sl < 2:
                            nc.sync.dma_start(
                                out=agin_a1[sl * 128:(sl + 1) * 128, :],
                                in_=aq[:, :])
                        else:
                            nc.sync.dma_start(out=agin_a2[:, :], in_=aq[:, :])
                        if grp == 0 and sl == 1:
                            nc.gpsimd.collective_compute(
                                "AllGather", mybir.AluOpType.bypass,
                                ins=[agin_a1.ap().opt()],
                                outs=[agout_a1.ap().opt()],
                                replica_groups=[list(range(NC))],
                            )
                    nc.gpsimd.collective_compute(
                        "AllGather", mybir.AluOpType.bypass,
                        ins=[(agin_b if grp == 1 else agin_a2).ap().opt()],
                        outs=[(agout_b if grp == 1 else agout_a2).ap().opt()],
                        replica_groups=[list(range(NC))],
                    )

                # ---- o_proj per token half (half B overlaps group-0
                # attention; half A accumulates slot-0/1 heads first so that
                # phase overlaps the small slot-2 AllGather)
                wo_by_ck = dict(zip(REAL_CHUNKS, wo_sb))
                CH01 = [j * 3 + s for j in range(NC) for s in range(2)]
                CH2 = [j * 3 + 2 for j in range(NC) if HEADS[j][2] is not None]
                for hf in (1, 0):
                    order = REAL_CHUNKS if hf == 1 else CH01 + CH2
                    agt = []
                    for ci, ck in enumerate(order):
                        t = agtp.tile([128, 512], f16, tag="agt", name="agt")
                        deng = nc.sync if ci % 2 == 0 else nc.scalar
                        if hf == 1:
                            src = agout_b[ck * 128:(ck + 1) * 128, :]
                        else:
                            cj, cs = divmod(ck, 3)
                            if cs < 2:
                                ro = cj * 256 + cs * 128
                                src = agout_a1[ro:ro + 128, :]
                            else:
                                src = agout_a2[cj * 128:(cj + 1) * 128, :]
                        deng.dma_start(out=t[:, :], in_=src)
                        agt.append(t)
                    for j in range(4):
                        m = hf * 4 + j
                        ps = pso.tile([128, OC], f32, tag="po", name="po")
                        for i, ck in enumerate(order):
                            nc.tensor.matmul(ps[:, :],
                                             agt[i][:, j * 128:(j + 1) * 128],
                                             wo_by_ck[ck][:, :],
                                             start=(i == 0),
                                             stop=(i == NH - 1))
                        og = ogp.tile([128, OC], f32, tag="og", name="og")
                        nc.scalar.copy(og[:, :], ps[:, :])
                        nc.sync.dma_start(out=out_d[m * 128:(m + 1) * 128, :],
                                          in_=og[:, :])

    nc.compile()
    return nc


def _host_prep(x, w_qkv, ws_qkv, w_o, ws_o, q_norm_w, k_norm_w):
    w_dq = (w_qkv * np.repeat(ws_qkv, GS, axis=1)).astype(np.float32)
    wo_dq = (w_o * np.repeat(ws_o, GS, axis=1)).astype(np.float32)

    pos = np.arange(S, dtype=np.float32)
    inv_freq = (THETA ** (-np.arange(0, HD, 2, dtype=np.float32) / HD)).astype(np.float32)
    ang = pos[:, None] * inv_freq[None, :]
    co = np.cos(ang).astype(np.float32)                  # [S, 64]
    si = np.sin(ang).astype(np.float32)
    # split rope tables for the d-permuted (even|odd) head layout:
    # t1 = [c*w_even | c*w_odd], t2 = [s*w_even | s*w_odd]
    tq1 = np.concatenate([co * q_norm_w[0::2], co * q_norm_w[1::2]], 1).astype(FP16)
    tq2 = np.concatenate([si * q_norm_w[0::2], si * q_norm_w[1::2]], 1).astype(FP16)
    tk1 = np.concatenate([co * k_norm_w[0::2], co * k_norm_w[1::2]], 1).astype(FP16)
    tk2 = np.concatenate([si * k_norm_w[0::2], si * k_norm_w[1::2]], 1).astype(FP16)
    # even dims then odd dims within each q/k head (scores are invariant
    # since q and k share the permutation; v / o_proj stay unpermuted)
    dperm = np.concatenate([np.arange(0, HD, 2), np.arange(1, HD, 2)])

    # mask variants: scoresT [k(128), 512 q]; group cols = 4 q-blocks; r = kc-4*grp
    cm = np.zeros((4, 128, 512), np.float32)
    tri = np.triu(np.ones((128, 128), np.float32))  # keep k <= q
    for r in range(4):
        for j in range(4):
            if j > r:
                cm[r, :, j * 128:(j + 1) * 128] = 1.0
            elif j == r:
                cm[r, :, j * 128:(j + 1) * 128] = tri
    cmask = cm.reshape(4 * 128, 512).astype(FP16)

    in_maps = []
    for c in range(NC):
        wq = np.zeros((DIM, WQCOLS), np.float32)
        for sl in range(3):
            h = HEADS[c][sl]
            if h is not None:
                wq[:, sl * 128:(sl + 1) * 128] = w_dq[h * HD + dperm, :].T
        ga = GA[c]
        wq[:, 384:512] = w_dq[KBASE + ga * HD + dperm, :].T
        wq[:, 512:640] = w_dq[VBASE + ga * HD:VBASE + (ga + 1) * HD, :].T
        gb = GB[c]
        if gb is not None:
            wq[:, 640:768] = w_dq[KBASE + gb * HD + dperm, :].T
            wq[:, 768:896] = w_dq[VBASE + gb * HD:VBASE + (gb + 1) * HD, :].T

        wo = np.zeros((NC * 384, OC), np.float32)
        for j in range(NC):
            for sl in range(3):
                h = HEADS[j][sl]
                if h is not None:
                    rws = slice((j * 3 + sl) * 128, (j * 3 + sl) * 128 + 128)
                    wo[rws, :] = wo_dq[c * OC:(c + 1) * OC, h * HD:(h + 1) * HD].T

        in_maps.append({
            "xs": x[c * 128:(c + 1) * 128].astype(np.float32),
            "wq": wq.astype(FP16),
            "wo": wo.astype(FP16),
            "tq1": tq1, "tq2": tq2, "tk1": tk1, "tk2": tk2,
            "cmask": cmask,
        })
    return in_maps


def kernel(x, w_qkv, ws_qkv, w_o, ws_o, q_norm_w, k_norm_w):
    x = np.asarray(x, np.float32)
    w_qkv = np.asarray(w_qkv, np.float32)
    ws_qkv = np.asarray(ws_qkv, np.float32)
    w_o = np.asarray(w_o, np.float32)
    ws_o = np.asarray(ws_o, np.float32)
    q_norm_w = np.asarray(q_norm_w, np.float32)
    k_norm_w = np.asarray(k_norm_w, np.float32)

    if "nc" not in _cached:
        _cached["nc"] = _build_nc()
    nc = _cached["nc"]

    in_maps = _host_prep(x, w_qkv, ws_qkv, w_o, ws_o, q_norm_w, k_norm_w)
    trace = bool(int(os.environ.get("BENCH_TRACE", "0")))
    res = run_bass_kernel_spmd(nc, in_maps, core_ids=list(range(NC)), trace=trace)
    _cached["res"] = res
    if trace and res.exec_time_ns is not None:
        print(f"HW exec time: {res.exec_time_ns} ns")
        _cached["exec_time_ns"] = res.exec_time_ns

    out = np.concatenate([np.asarray(res.results[c]["out"], np.float32)
                          for c in range(NC)], axis=1)
    return out

